# revision 24
# baseline (speedup 1.0000x reference)
"""LIF activation (hard-reset leaky integrate-and-fire) on 8 Trainium2 cores.

Math (per lane, per step t):
    u_t   = x_t + z_{t-1}
    Vm_t  = relu(u_t)
    keep  = 1{Vm_t < 1}  == 1{u_t < 1}
    z_t   = (1 - w_leak) * Vm_t * keep     (carried state, pre-scaled)
    spike = 1{u_t > 1}                     (strict >, the output)

Engines (all exact f32; lanes are independent, so the channel dim is split
between two engines that each run their own serial recurrence):
  DVE   (channels [0:CD]):   u = x + z                  tensor_tensor add
                             t1 = 1{u<1} * u            scalar_tensor_tensor
                             z = max(t1,0) * W1         scalar_tensor_tensor
  Pool  (channels [CD:C]):   u = x + z                  tensor_tensor add
                             s = 1{u>1}                 tensor_scalar is_gt
                             c = clip(u,0,1)            tensor_scalar min,max
                             g = 1{u<1}                 tensor_scalar is_lt
                             t1 = c * g                 tensor_tensor mult
                             z = t1 * W1                tensor_tensor mult
  Act   (DVE's channels):    s = Sign(u - 1)            activation, off-path
The spike is recovered on the host as (s > 0); Sign/is_gt are exact in f32,
so the result is bit-identical to the f32 reference. Emitting s in fp16
(values in {-1, 0, 1} are exact) halves the output DMA.

Sharding: time is split unevenly across the 8 cores: every core computes
T_IN = 132 steps; core 0 outputs all 132 (starts from the true z=0), cores
1-7 warm up speculatively for WARM=8 steps from z=0 and output 124 steps.
Hard resets make any starting state collapse: if within the warmup window a
lane sees x_t >= 1 (forced reset for any state) or x_t <= -(1-w_leak)
(forced relu clamp), the warmed-up state is provably bit-exact. Lanes with
no such certificate in some window are recomputed exactly on the host and
patched in, so the result is exact for any input.
"""
import numpy as np
import sys

for _p in ("/opt/trn_rl_repo",):
    if _p not in sys.path:
        sys.path.append(_p)

import concourse.bass as bass
import concourse.mybir as mybir
from concourse.tile import TileContext
from concourse import bass_utils

F32 = mybir.dt.float32
F16 = mybir.dt.float16
OP = mybir.AluOpType
AF = mybir.ActivationFunctionType

B, T, C = 128, 1000, 512
NCORES = 8
WARM = 8                  # speculative warmup steps (cores 1..7)
T_IN = (T + (NCORES - 1) * WARM) // NCORES   # 132 compute steps per core
L = T_IN - WARM           # 124 output steps per core 1..7; core 0 emits T_IN
CHUNK = 16                # time steps per DMA chunk
CHUNK0 = 4                # ramp unit for leading/trailing chunks
CD = 424                  # channels on DVE; C - CD on Pool
LAGP = 4                  # uP rotation depth headroom (see uPs below)

TRACE = False             # kept for test.py compatibility (no NTFF here)
LAST_RESULTS = None       # BassKernelResults of the last run


def in_chunk_sizes(t_in=T_IN, chunk=CHUNK, chunk0=CHUNK0):
    """Ramped chunk plan: small leading chunks so compute starts before the
    full-size DMAs land, and small trailing chunks so the final stores are
    short. E.g. t_in=132 -> [1, 2, 4, 8, 16*6, 13, 4, 4]."""
    sizes = [1, chunk0 // 2, chunk0, 2 * chunk0] if chunk0 else []
    o = sum(sizes)
    while t_in - o > chunk + 2 * chunk0:
        sizes.append(chunk)
        o += chunk
    rem = t_in - o
    if rem > 2 * chunk0:
        sizes += [rem - 2 * chunk0, chunk0, chunk0]
    elif rem:
        sizes.append(rem)
    return sizes


def lif_body(tc, out_aps, x_ap, w1_ap, t_in=T_IN, cd=CD):
    """Emit the per-core LIF program.

    out_aps: list of [B, csz_i, C] f16 DRAM tensors, one per chunk
    x_ap: [B, t_in, C] f32 DRAM
    w1_ap:  [128, C] f32 DRAM (1 - w_leak, pre-broadcast over partitions)
    """
    nc = tc.nc
    cp = C - cd
    csizes = in_chunk_sizes(t_in)
    with tc.tile_pool(name="const", bufs=1) as constp, \
         tc.tile_pool(name="state", bufs=1) as statep, \
         tc.tile_pool(name="xin", bufs=3) as xinp, \
         tc.tile_pool(name="outs", bufs=3) as outp:
        w1t = constp.tile([128, C], F32)
        nc.sync.dma_start(out=w1t, in_=w1_ap)
        bias = constp.tile([128, 1], F32)
        nc.vector.memset(bias, -1.0)
        # --- DVE-slice state: two independent half-chains (A|B) interleaved
        # so each op's RAW producer is two instructions back and the DVE
        # pipeline's side-effect latency is hidden ---
        ca = cd // 2
        zA = statep.tile([128, ca], F32)
        nc.vector.memset(zA, 0.0)
        zB = statep.tile([128, cd - ca], F32)
        nc.vector.memset(zB, 0.0)
        t1A = statep.tile([128, ca], F32, tag="t1A")
        t1B = statep.tile([128, cd - ca], F32, tag="t1B")
        # Act reads u; rotate deep so the cross-engine WAR wait is stale by
        # the time the DVE add reuses a slot
        uAs = [statep.tile([128, ca], F32, name=f"uA{i}", tag=f"uA{i}")
               for i in range(6)]
        uBs = [statep.tile([128, cd - ca], F32, name=f"uB{i}", tag=f"uB{i}")
               for i in range(6)]
        # --- Pool-slice state: Pool's spike op reads uP one step late, so
        # rotate uP a few slots deep ---
        zP = statep.tile([128, cp], F32)
        nc.gpsimd.memset(zP, 0.0)
        cP = statep.tile([128, cp], F32, tag="cP")
        gP = statep.tile([128, cp], F32, tag="gP")
        t1P = statep.tile([128, cp], F32, tag="t1P")
        uPs = [statep.tile([128, cp], F32, name=f"uP{i}", tag=f"uP{i}")
               for i in range(LAGP + 6)]
        # 1-element scratch: "touch" ops read freshly-DMA'd tiles so the
        # DMA-completion wait lands on the touch, not on a compute op (the
        # compute ISA structs have one sync-wait slot and the serial RAW
        # chain already consumes it).
        touch_w = statep.tile([128, 1], F32, tag="touch_w")
        nc.vector.tensor_copy(out=touch_w, in_=w1t[:, :1])
        touch_wp = statep.tile([128, 1], F32, tag="touch_wp")
        nc.gpsimd.tensor_copy(out=touch_wp, in_=w1t[:, :1])

        # step t -> (out_tile, offset); Pool's spike for step t is emitted one
        # iteration late so it fills the dependency bubble after add(t+1)
        # instead of sitting between add(t) and its dependents
        spike_slot = {}
        pend_out = {}

        def emit_spikeP(tp):
            otile, offp, icp = spike_slot.pop(tp)
            nc.gpsimd.tensor_scalar(out=otile[:, offp, cd:],
                                    in0=uPs[tp % len(uPs)],
                                    scalar1=1.0, scalar2=None, op0=OP.is_gt)
            if icp is not None:
                nc.sync.dma_start(out=out_aps[icp], in_=pend_out.pop(icp))

        s = 0
        for ic, csz in enumerate(csizes):
            xin = xinp.tile([128, csz, C], F32, tag="xin")
            nc.sync.dma_start(out=xin, in_=x_ap[:, s:s + csz, :])
            tchD = statep.tile([128, 1], F32, tag=f"touchD{ic}")
            nc.vector.tensor_copy(out=tchD, in_=xin[:, 0, :1])
            tchP = statep.tile([128, 1], F32, tag=f"touchP{ic}")
            nc.gpsimd.tensor_copy(out=tchP, in_=xin[:, 0, cd:cd + 1])
            out_tile = outp.tile([128, csz, C], F16, tag="out")
            pend_out[ic] = out_tile

            for off in range(csz):
                t = s + off
                uA = uAs[t % 6]
                uB = uBs[t % 6]
                uP = uPs[t % len(uPs)]
                # DVE slice: 3-op chain x 2 interleaved half-chains
                nc.vector.tensor_add(out=uA, in0=xin[:, off, :ca], in1=zA)
                nc.vector.tensor_add(out=uB, in0=xin[:, off, ca:cd], in1=zB)
                nc.vector.scalar_tensor_tensor(
                    out=t1A, in0=uA, scalar=1.0, in1=uA,
                    op0=OP.is_lt, op1=OP.mult)
                nc.vector.scalar_tensor_tensor(
                    out=t1B, in0=uB, scalar=1.0, in1=uB,
                    op0=OP.is_lt, op1=OP.mult)
                nc.vector.scalar_tensor_tensor(
                    out=zA, in0=t1A, scalar=0.0, in1=w1t[:, :ca],
                    op0=OP.max, op1=OP.mult)
                nc.vector.scalar_tensor_tensor(
                    out=zB, in0=t1B, scalar=0.0, in1=w1t[:, ca:cd],
                    op0=OP.max, op1=OP.mult)
                # Pool slice: 6-op chain (walrus rejects STT on Pool); Pool
                # emits its own spikes so the two chains never couple through
                # the Act queue, and one step late so the spike op fills the
                # z->add dependency bubble
                nc.gpsimd.tensor_tensor(out=uP, in0=xin[:, off, cd:], in1=zP,
                                        op=OP.add)
                spike_slot[t] = (out_tile, off,
                                 ic if off == csz - 1 else None)
                if t - 1 in spike_slot:
                    emit_spikeP(t - 1)
                nc.gpsimd.tensor_scalar(out=cP, in0=uP, scalar1=1.0,
                                        scalar2=0.0, op0=OP.min, op1=OP.max)
                nc.gpsimd.tensor_scalar(out=gP, in0=uP, scalar1=1.0,
                                        scalar2=None, op0=OP.is_lt)
                nc.gpsimd.tensor_tensor(out=t1P, in0=cP, in1=gP, op=OP.mult)
                nc.gpsimd.tensor_tensor(out=zP, in0=t1P, in1=w1t[:, cd:],
                                        op=OP.mult)
                # spike carrier: s = Sign(u - 1) on Act for the DVE slice
                nc.scalar.activation(out=out_tile[:, off, :ca], in_=uA,
                                     func=AF.Sign, bias=bias[:], scale=1.0)
                nc.scalar.activation(out=out_tile[:, off, ca:cd], in_=uB,
                                     func=AF.Sign, bias=bias[:], scale=1.0)

            if ic < len(csizes) - 1:
                # release markers: a 4-byte write by each engine after its
                # last read of the slot, so the refill DMA's waits resolve to
                # one sem per engine (transitively covering the old DMA).
                nc.vector.memset(xin[:, 0, :1], 0.0)
                nc.gpsimd.memset(xin[:, 0, cd:cd + 1], 0.0)
            s += csz

        emit_spikeP(t_in - 1)


def _legalize_waits(nc):
    """Walrus accepts at most one sync wait on compute/DMA ISA structs.
    Split extra waits onto standalone EventSemaphore instructions inserted
    just before, on the same engine queue (identical blocking semantics)."""
    import bass_rust
    skip = ("InstEventSemaphore",)
    for f in nc.m.functions:
        for bb in f.blocks:
            insts = bb.instructions
            k = 0
            while k < len(insts):
                i = insts[k]
                si = i.sync_info
                if (si is not None and si.on_wait and len(si.on_wait) > 1
                        and type(i).__name__ not in skip):
                    waits = list(si.on_wait)
                    for j, w in enumerate(waits[:-1]):
                        ev = mybir.InstEventSemaphore(
                            name=f"{i.name}-evw{j}",
                            engine=i.engine,
                            ins=[], outs=[],
                            sync_info=bass_rust.SyncInfo(
                                on_wait=[w], on_update=[]),
                        )
                        insts.insert(k, ev)
                        k += 1
                    i.sync_info = bass_rust.SyncInfo(
                        on_wait=[waits[-1]], on_update=si.on_update)
                k += 1


def build(t_in=T_IN, cd=CD):
    nc = bass.Bass("TRN2", target_bir_lowering=False, debug=False,
                   enable_asserts=False, num_devices=NCORES)
    x_d = nc.dram_tensor("x_local", [B, t_in, C], F32, kind="ExternalInput")
    w1_d = nc.dram_tensor("w1b", [128, C], F32, kind="ExternalInput")
    out_ds = [
        nc.dram_tensor(f"spikes{i}", [B, csz, C], F16, kind="ExternalOutput")
        for i, csz in enumerate(in_chunk_sizes(t_in))
    ]
    with TileContext(nc) as tc:
        lif_body(tc, [d[:] for d in out_ds], x_d[:], w1_d[:], t_in, cd)
    _legalize_waits(nc)
    return nc


def _core_start(k):
    """Global t of core k's first OUTPUT step."""
    return 0 if k == 0 else T_IN + (k - 1) * L


def _host_repair(out, x, w1):
    """Exactly recompute lanes whose warmup windows lack a reset/clamp
    certificate at some core boundary, and patch them into `out`."""
    missing = np.zeros((B, C), bool)
    for k in range(1, NCORES):
        t0 = _core_start(k)
        win = x[:, t0 - WARM:t0, :]
        cert = ((win >= np.float32(1.0)) |
                (win <= -w1[None, None, :])).any(axis=1)
        missing |= ~cert
    if not missing.any():
        return 0
    bb, cc = np.nonzero(missing)
    xs = x[bb, :, cc]                     # [R, T]
    a = w1[cc]                            # [R]
    zz = np.zeros(len(bb), np.float32)
    one = np.float32(1.0)
    zero = np.float32(0.0)
    sp = np.empty((len(bb), T), np.float32)
    for t in range(T):
        u = (xs[:, t] + zz).astype(np.float32)
        t1 = ((u < one).astype(np.float32) * u).astype(np.float32)
        zz = (np.maximum(t1, zero) * a).astype(np.float32)
        sp[:, t] = (u > one).astype(np.float32)
    out[bb, :, cc] = sp
    return len(bb)


def kernel(x, w_leak):
    global LAST_RESULTS
    x = np.ascontiguousarray(np.asarray(x), dtype=np.float32)
    w_leak = np.ascontiguousarray(np.asarray(w_leak), dtype=np.float32)
    w1 = (np.float32(1.0) - w_leak).astype(np.float32)       # [C]
    w1b = np.ascontiguousarray(np.broadcast_to(w1[None, :], (128, C)),
                               dtype=np.float32)

    in_maps = []
    for k in range(NCORES):
        t0 = _core_start(k)
        lo = t0 - (0 if k == 0 else WARM)
        in_maps.append({
            "x_local": np.ascontiguousarray(x[:, lo:lo + T_IN, :]),
            "w1b": w1b,
        })

    nc = build()
    res = bass_utils.run_bass_kernel_spmd(
        nc, in_maps, core_ids=list(range(NCORES)), trace=TRACE)
    LAST_RESULTS = res
    nchunks = len(in_chunk_sizes(T_IN))
    out = np.empty((B, T, C), np.float32)
    for k in range(NCORES):
        sgn = np.concatenate(
            [res.results[k][f"spikes{i}"] for i in range(nchunks)], axis=1)
        skip = 0 if k == 0 else WARM
        t0 = _core_start(k)
        n = T_IN - skip
        out[:, t0:t0 + n, :] = (sgn[:, skip:, :] > 0).astype(np.float32)
    _host_repair(out, x, w1)
    return out


# revision 27
# speedup vs baseline: 1.0050x; 1.0050x over previous
"""LIF activation (hard-reset leaky integrate-and-fire) on 8 Trainium2 cores.

Math (per lane, per step t):
    u_t   = x_t + z_{t-1}
    Vm_t  = relu(u_t)
    keep  = 1{Vm_t < 1}  == 1{u_t < 1}
    z_t   = (1 - w_leak) * Vm_t * keep     (carried state, pre-scaled)
    spike = 1{u_t > 1}                     (strict >, the output)

Engines (all exact f32; lanes are independent, so the channel dim is split
between two engines that each run their own serial recurrence):
  DVE   (channels [0:CD]):   u = x + z                  tensor_tensor add
                             t1 = 1{u<1} * u            scalar_tensor_tensor
                             z = max(t1,0) * W1         scalar_tensor_tensor
  Pool  (channels [CD:C]):   u = x + z                  tensor_tensor add
                             s = 1{u>1}                 tensor_scalar is_gt
                             c = clip(u,0,1)            tensor_scalar min,max
                             g = 1{u<1}                 tensor_scalar is_lt
                             t1 = c * g                 tensor_tensor mult
                             z = t1 * W1                tensor_tensor mult
  Act   (DVE's channels):    s = Sign(u - 1)            activation, off-path
The spike is recovered on the host as (s > 0); Sign/is_gt are exact in f32,
so the result is bit-identical to the f32 reference. Emitting s in fp16
(values in {-1, 0, 1} are exact) halves the output DMA.

Sharding: time is split unevenly across the 8 cores: every core computes
T_IN = 132 steps; core 0 outputs all 132 (starts from the true z=0), cores
1-7 warm up speculatively for WARM=8 steps from z=0 and output 124 steps.
Hard resets make any starting state collapse: if within the warmup window a
lane sees x_t >= 1 (forced reset for any state) or x_t <= -(1-w_leak)
(forced relu clamp), the warmed-up state is provably bit-exact. Lanes with
no such certificate in some window are recomputed exactly on the host and
patched in, so the result is exact for any input.
"""
import numpy as np
import sys

for _p in ("/opt/trn_rl_repo",):
    if _p not in sys.path:
        sys.path.append(_p)

import concourse.bass as bass
import concourse.mybir as mybir
from concourse.tile import TileContext
from concourse import bass_utils

F32 = mybir.dt.float32
F16 = mybir.dt.float16
OP = mybir.AluOpType
AF = mybir.ActivationFunctionType

B, T, C = 128, 1000, 512
NCORES = 8
WARM = 8                  # speculative warmup steps (cores 1..7)
T_IN = (T + (NCORES - 1) * WARM) // NCORES   # 132 compute steps per core
L = T_IN - WARM           # 124 output steps per core 1..7; core 0 emits T_IN
CHUNK = 16                # time steps per DMA chunk
CHUNK0 = 4                # ramp unit for leading/trailing chunks
CD = 424                  # channels on DVE; C - CD on Pool
LAGP = 4                  # uP rotation depth headroom (see uPs below)

TRACE = False             # kept for test.py compatibility (no NTFF here)
LAST_RESULTS = None       # BassKernelResults of the last run


def in_chunk_sizes(t_in=T_IN, chunk=CHUNK, chunk0=CHUNK0):
    """Ramped chunk plan: small leading chunks so compute starts before the
    full-size DMAs land, and small trailing chunks so the final stores are
    short. E.g. t_in=132 -> [1, 2, 4, 8, 16*6, 13, 4, 4]."""
    sizes = [1, chunk0 // 2, chunk0, 2 * chunk0] if chunk0 else []
    o = sum(sizes)
    while t_in - o > chunk + 2 * chunk0:
        sizes.append(chunk)
        o += chunk
    rem = t_in - o
    if rem > 3 * chunk0:
        sizes += [rem - 3 * chunk0, chunk0, chunk0, chunk0 // 2, chunk0 // 2]
    elif rem:
        sizes.append(rem)
    return sizes


def lif_body(tc, out_aps, x_ap, w1_ap, t_in=T_IN, cd=CD):
    """Emit the per-core LIF program.

    out_aps: list of [B, csz_i, C] f16 DRAM tensors, one per chunk
    x_ap: [B, t_in, C] f32 DRAM
    w1_ap:  [128, C] f32 DRAM (1 - w_leak, pre-broadcast over partitions)
    """
    nc = tc.nc
    cp = C - cd
    csizes = in_chunk_sizes(t_in)
    with tc.tile_pool(name="const", bufs=1) as constp, \
         tc.tile_pool(name="state", bufs=1) as statep, \
         tc.tile_pool(name="xin", bufs=3) as xinp, \
         tc.tile_pool(name="outs", bufs=3) as outp:
        # first x chunk's DMA goes ahead of everything else on the (serialized)
        # DMA engines — it gates the first compute step; w1 is needed two ops
        # later and can follow
        xin0 = xinp.tile([128, csizes[0], C], F32, tag="xin")
        nc.sync.dma_start(out=xin0, in_=x_ap[:, 0:csizes[0], :])
        w1t = constp.tile([128, C], F32)
        nc.sync.dma_start(out=w1t, in_=w1_ap)
        bias = constp.tile([128, 1], F32)
        nc.vector.memset(bias, -1.0)
        # --- DVE-slice state: two independent half-chains (A|B) interleaved
        # so each op's RAW producer is two instructions back and the DVE
        # pipeline's side-effect latency is hidden ---
        ca = cd // 2
        zA = statep.tile([128, ca], F32)
        nc.vector.memset(zA, 0.0)
        zB = statep.tile([128, cd - ca], F32)
        nc.vector.memset(zB, 0.0)
        t1A = statep.tile([128, ca], F32, tag="t1A")
        t1B = statep.tile([128, cd - ca], F32, tag="t1B")
        # Act reads u; rotate deep so the cross-engine WAR wait is stale by
        # the time the DVE add reuses a slot
        uAs = [statep.tile([128, ca], F32, name=f"uA{i}", tag=f"uA{i}")
               for i in range(6)]
        uBs = [statep.tile([128, cd - ca], F32, name=f"uB{i}", tag=f"uB{i}")
               for i in range(6)]
        # --- Pool-slice state: Pool's spike op reads uP one step late, so
        # rotate uP a few slots deep ---
        zP = statep.tile([128, cp], F32)
        nc.gpsimd.memset(zP, 0.0)
        cP = statep.tile([128, cp], F32, tag="cP")
        gP = statep.tile([128, cp], F32, tag="gP")
        t1P = statep.tile([128, cp], F32, tag="t1P")
        uPs = [statep.tile([128, cp], F32, name=f"uP{i}", tag=f"uP{i}")
               for i in range(LAGP + 6)]
        # 1-element scratch: "touch" ops read freshly-DMA'd tiles so the
        # DMA-completion wait lands on the touch, not on a compute op (the
        # compute ISA structs have one sync-wait slot and the serial RAW
        # chain already consumes it).
        touch_w = statep.tile([128, 1], F32, tag="touch_w")
        nc.vector.tensor_copy(out=touch_w, in_=w1t[:, :1])
        touch_wp = statep.tile([128, 1], F32, tag="touch_wp")
        nc.gpsimd.tensor_copy(out=touch_wp, in_=w1t[:, :1])

        # step t -> (out_tile, offset); Pool's spike for step t is emitted one
        # iteration late so it fills the dependency bubble after add(t+1)
        # instead of sitting between add(t) and its dependents
        spike_slot = {}
        pend_out = {}

        def emit_spikeP(tp):
            otile, offp, icp = spike_slot.pop(tp)
            nc.gpsimd.tensor_scalar(out=otile[:, offp, cd:],
                                    in0=uPs[tp % len(uPs)],
                                    scalar1=1.0, scalar2=None, op0=OP.is_gt)
            if icp is not None:
                nc.sync.dma_start(out=out_aps[icp], in_=pend_out.pop(icp))

        s = 0
        for ic, csz in enumerate(csizes):
            if ic == 0:
                xin = xin0
            else:
                xin = xinp.tile([128, csz, C], F32, tag="xin")
                nc.sync.dma_start(out=xin, in_=x_ap[:, s:s + csz, :])
            tchD = statep.tile([128, 1], F32, tag=f"touchD{ic}")
            nc.vector.tensor_copy(out=tchD, in_=xin[:, 0, :1])
            tchP = statep.tile([128, 1], F32, tag=f"touchP{ic}")
            nc.gpsimd.tensor_copy(out=tchP, in_=xin[:, 0, cd:cd + 1])
            out_tile = outp.tile([128, csz, C], F16, tag="out")
            pend_out[ic] = out_tile

            for off in range(csz):
                t = s + off
                uA = uAs[t % 6]
                uB = uBs[t % 6]
                uP = uPs[t % len(uPs)]
                # DVE slice: 3-op chain x 2 interleaved half-chains
                nc.vector.tensor_add(out=uA, in0=xin[:, off, :ca], in1=zA)
                nc.vector.tensor_add(out=uB, in0=xin[:, off, ca:cd], in1=zB)
                nc.vector.scalar_tensor_tensor(
                    out=t1A, in0=uA, scalar=1.0, in1=uA,
                    op0=OP.is_lt, op1=OP.mult)
                nc.vector.scalar_tensor_tensor(
                    out=t1B, in0=uB, scalar=1.0, in1=uB,
                    op0=OP.is_lt, op1=OP.mult)
                nc.vector.scalar_tensor_tensor(
                    out=zA, in0=t1A, scalar=0.0, in1=w1t[:, :ca],
                    op0=OP.max, op1=OP.mult)
                nc.vector.scalar_tensor_tensor(
                    out=zB, in0=t1B, scalar=0.0, in1=w1t[:, ca:cd],
                    op0=OP.max, op1=OP.mult)
                # Pool slice: 6-op chain (walrus rejects STT on Pool); Pool
                # emits its own spikes so the two chains never couple through
                # the Act queue, and one step late so the spike op fills the
                # z->add dependency bubble
                nc.gpsimd.tensor_tensor(out=uP, in0=xin[:, off, cd:], in1=zP,
                                        op=OP.add)
                spike_slot[t] = (out_tile, off,
                                 ic if off == csz - 1 else None)
                if t - 1 in spike_slot:
                    emit_spikeP(t - 1)
                nc.gpsimd.tensor_scalar(out=cP, in0=uP, scalar1=1.0,
                                        scalar2=0.0, op0=OP.min, op1=OP.max)
                nc.gpsimd.tensor_scalar(out=gP, in0=uP, scalar1=1.0,
                                        scalar2=None, op0=OP.is_lt)
                nc.gpsimd.tensor_tensor(out=t1P, in0=cP, in1=gP, op=OP.mult)
                nc.gpsimd.tensor_tensor(out=zP, in0=t1P, in1=w1t[:, cd:],
                                        op=OP.mult)
                # spike carrier: s = Sign(u - 1) on Act for the DVE slice
                nc.scalar.activation(out=out_tile[:, off, :ca], in_=uA,
                                     func=AF.Sign, bias=bias[:], scale=1.0)
                nc.scalar.activation(out=out_tile[:, off, ca:cd], in_=uB,
                                     func=AF.Sign, bias=bias[:], scale=1.0)

            if ic < len(csizes) - 1:
                # release markers: a 4-byte write by each engine after its
                # last read of the slot, so the refill DMA's waits resolve to
                # one sem per engine (transitively covering the old DMA).
                nc.vector.memset(xin[:, 0, :1], 0.0)
                nc.gpsimd.memset(xin[:, 0, cd:cd + 1], 0.0)
            s += csz

        emit_spikeP(t_in - 1)


def _legalize_waits(nc):
    """Walrus accepts at most one sync wait on compute/DMA ISA structs.
    Split extra waits onto standalone EventSemaphore instructions inserted
    just before, on the same engine queue (identical blocking semantics)."""
    import bass_rust
    skip = ("InstEventSemaphore",)
    for f in nc.m.functions:
        for bb in f.blocks:
            insts = bb.instructions
            k = 0
            while k < len(insts):
                i = insts[k]
                si = i.sync_info
                if (si is not None and si.on_wait and len(si.on_wait) > 1
                        and type(i).__name__ not in skip):
                    waits = list(si.on_wait)
                    for j, w in enumerate(waits[:-1]):
                        ev = mybir.InstEventSemaphore(
                            name=f"{i.name}-evw{j}",
                            engine=i.engine,
                            ins=[], outs=[],
                            sync_info=bass_rust.SyncInfo(
                                on_wait=[w], on_update=[]),
                        )
                        insts.insert(k, ev)
                        k += 1
                    i.sync_info = bass_rust.SyncInfo(
                        on_wait=[waits[-1]], on_update=si.on_update)
                k += 1


def build(t_in=T_IN, cd=CD):
    nc = bass.Bass("TRN2", target_bir_lowering=False, debug=False,
                   enable_asserts=False, num_devices=NCORES)
    x_d = nc.dram_tensor("x_local", [B, t_in, C], F32, kind="ExternalInput")
    w1_d = nc.dram_tensor("w1b", [128, C], F32, kind="ExternalInput")
    out_ds = [
        nc.dram_tensor(f"spikes{i}", [B, csz, C], F16, kind="ExternalOutput")
        for i, csz in enumerate(in_chunk_sizes(t_in))
    ]
    with TileContext(nc) as tc:
        lif_body(tc, [d[:] for d in out_ds], x_d[:], w1_d[:], t_in, cd)
    _legalize_waits(nc)
    return nc


def _core_start(k):
    """Global t of core k's first OUTPUT step."""
    return 0 if k == 0 else T_IN + (k - 1) * L


def _host_repair(out, x, w1):
    """Exactly recompute lanes whose warmup windows lack a reset/clamp
    certificate at some core boundary, and patch them into `out`."""
    missing = np.zeros((B, C), bool)
    for k in range(1, NCORES):
        t0 = _core_start(k)
        win = x[:, t0 - WARM:t0, :]
        cert = ((win >= np.float32(1.0)) |
                (win <= -w1[None, None, :])).any(axis=1)
        missing |= ~cert
    if not missing.any():
        return 0
    bb, cc = np.nonzero(missing)
    xs = x[bb, :, cc]                     # [R, T]
    a = w1[cc]                            # [R]
    zz = np.zeros(len(bb), np.float32)
    one = np.float32(1.0)
    zero = np.float32(0.0)
    sp = np.empty((len(bb), T), np.float32)
    for t in range(T):
        u = (xs[:, t] + zz).astype(np.float32)
        t1 = ((u < one).astype(np.float32) * u).astype(np.float32)
        zz = (np.maximum(t1, zero) * a).astype(np.float32)
        sp[:, t] = (u > one).astype(np.float32)
    out[bb, :, cc] = sp
    return len(bb)


def kernel(x, w_leak):
    global LAST_RESULTS
    x = np.ascontiguousarray(np.asarray(x), dtype=np.float32)
    w_leak = np.ascontiguousarray(np.asarray(w_leak), dtype=np.float32)
    w1 = (np.float32(1.0) - w_leak).astype(np.float32)       # [C]
    w1b = np.ascontiguousarray(np.broadcast_to(w1[None, :], (128, C)),
                               dtype=np.float32)

    in_maps = []
    for k in range(NCORES):
        t0 = _core_start(k)
        lo = t0 - (0 if k == 0 else WARM)
        in_maps.append({
            "x_local": np.ascontiguousarray(x[:, lo:lo + T_IN, :]),
            "w1b": w1b,
        })

    nc = build()
    res = bass_utils.run_bass_kernel_spmd(
        nc, in_maps, core_ids=list(range(NCORES)), trace=TRACE)
    LAST_RESULTS = res
    nchunks = len(in_chunk_sizes(T_IN))
    out = np.empty((B, T, C), np.float32)
    for k in range(NCORES):
        sgn = np.concatenate(
            [res.results[k][f"spikes{i}"] for i in range(nchunks)], axis=1)
        skip = 0 if k == 0 else WARM
        t0 = _core_start(k)
        n = T_IN - skip
        out[:, t0:t0 + n, :] = (sgn[:, skip:, :] > 0).astype(np.float32)
    _host_repair(out, x, w1)
    return out


# revision 30
# speedup vs baseline: 1.0059x; 1.0009x over previous
"""LIF activation (hard-reset leaky integrate-and-fire) on 8 Trainium2 cores.

Math (per lane, per step t):
    u_t   = x_t + z_{t-1}
    Vm_t  = relu(u_t)
    keep  = 1{Vm_t < 1}  == 1{u_t < 1}
    z_t   = (1 - w_leak) * Vm_t * keep     (carried state, pre-scaled)
    spike = 1{u_t > 1}                     (strict >, the output)

Engines (all exact f32; lanes are independent, so the channel dim is split
between two engines that each run their own serial recurrence):
  DVE   (channels [0:CD]):   u = x + z                  tensor_tensor add
                             t1 = 1{u<1} * u            scalar_tensor_tensor
                             z = max(t1,0) * W1         scalar_tensor_tensor
  Pool  (channels [CD:C]):   u = x + z                  tensor_tensor add
                             s = 1{u>1}                 tensor_scalar is_gt
                             c = clip(u,0,1)            tensor_scalar min,max
                             g = 1{u<1}                 tensor_scalar is_lt
                             t1 = c * g                 tensor_tensor mult
                             z = t1 * W1                tensor_tensor mult
  Act   (DVE's channels):    s = Sign(u - 1)            activation, off-path
The spike is recovered on the host as (s > 0); Sign/is_gt are exact in f32,
so the result is bit-identical to the f32 reference. Emitting s in fp16
(values in {-1, 0, 1} are exact) halves the output DMA.

Sharding: time is split unevenly across the 8 cores: every core computes
T_IN = 132 steps; core 0 outputs all 132 (starts from the true z=0), cores
1-7 warm up speculatively for WARM=8 steps from z=0 and output 124 steps.
Hard resets make any starting state collapse: if within the warmup window a
lane sees x_t >= 1 (forced reset for any state) or x_t <= -(1-w_leak)
(forced relu clamp), the warmed-up state is provably bit-exact. Lanes with
no such certificate in some window are recomputed exactly on the host and
patched in, so the result is exact for any input.
"""
import numpy as np
import sys

for _p in ("/opt/trn_rl_repo",):
    if _p not in sys.path:
        sys.path.append(_p)

import concourse.bass as bass
import concourse.mybir as mybir
from concourse.tile import TileContext
from concourse import bass_utils

F32 = mybir.dt.float32
F16 = mybir.dt.float16
OP = mybir.AluOpType
AF = mybir.ActivationFunctionType

B, T, C = 128, 1000, 512
NCORES = 8
WARM = 8                  # speculative warmup steps (cores 1..7)
T_IN = (T + (NCORES - 1) * WARM) // NCORES   # 132 compute steps per core
L = T_IN - WARM           # 124 output steps per core 1..7; core 0 emits T_IN
CHUNK = 16                # time steps per DMA chunk
CHUNK0 = 4                # ramp unit for leading/trailing chunks
CD = 424                  # channels on DVE; C - CD on Pool
LAGP = 4                  # uP rotation depth headroom (see uPs below)

TRACE = False             # kept for test.py compatibility (no NTFF here)
LAST_RESULTS = None       # BassKernelResults of the last run


def in_chunk_sizes(t_in=T_IN, chunk=CHUNK, chunk0=CHUNK0):
    """Ramped chunk plan: small leading chunks so compute starts before the
    full-size DMAs land, and small trailing chunks so the final stores are
    short. E.g. t_in=132 -> [1, 2, 4, 8, 16*6, 13, 4, 4]."""
    sizes = [1, chunk0 // 2, chunk0, 2 * chunk0] if chunk0 else []
    o = sum(sizes)
    while t_in - o > chunk + 2 * chunk0:
        sizes.append(chunk)
        o += chunk
    rem = t_in - o
    if rem > 3 * chunk0:
        sizes += [rem - 3 * chunk0, chunk0, chunk0, chunk0 // 2, chunk0 // 2]
    elif rem:
        sizes.append(rem)
    return sizes


def lif_body(tc, out_aps, x_ap, w1_ap, t_in=T_IN, cd=CD):
    """Emit the per-core LIF program.

    out_aps: list of [B, csz_i, C] f16 DRAM tensors, one per chunk
    x_ap: [B, t_in, C] f32 DRAM
    w1_ap:  [128, C] f32 DRAM (1 - w_leak, pre-broadcast over partitions)
    """
    nc = tc.nc
    cp = C - cd
    csizes = in_chunk_sizes(t_in)
    with tc.tile_pool(name="const", bufs=1) as constp, \
         tc.tile_pool(name="state", bufs=1) as statep, \
         tc.tile_pool(name="xin", bufs=3) as xinp, \
         tc.tile_pool(name="outs", bufs=3) as outp:
        # first x chunk's DMA goes ahead of everything else on the (serialized)
        # DMA engines — it gates the first compute step; w1 is needed two ops
        # later and can follow
        xin0 = xinp.tile([128, csizes[0], C], F32, tag="xin")
        nc.sync.dma_start(out=xin0, in_=x_ap[:, 0:csizes[0], :])
        w1t = constp.tile([128, C], F32)
        nc.sync.dma_start(out=w1t, in_=w1_ap)
        bias = constp.tile([128, 1], F32)
        nc.vector.memset(bias, -1.0)
        # --- DVE-slice state: two independent half-chains (A|B) interleaved
        # so each op's RAW producer is two instructions back and the DVE
        # pipeline's side-effect latency is hidden ---
        ca = cd // 2
        zA = statep.tile([128, ca], F32)
        nc.vector.memset(zA, 0.0)
        zB = statep.tile([128, cd - ca], F32)
        nc.vector.memset(zB, 0.0)
        t1A = statep.tile([128, ca], F32, tag="t1A")
        t1B = statep.tile([128, cd - ca], F32, tag="t1B")
        # Act reads u; rotate deep so the cross-engine WAR wait is stale by
        # the time the DVE add reuses a slot
        uAs = [statep.tile([128, ca], F32, name=f"uA{i}", tag=f"uA{i}")
               for i in range(6)]
        uBs = [statep.tile([128, cd - ca], F32, name=f"uB{i}", tag=f"uB{i}")
               for i in range(6)]
        # --- Pool-slice state: Pool's spike op reads uP one step late, so
        # rotate uP a few slots deep ---
        zP = statep.tile([128, cp], F32)
        nc.gpsimd.memset(zP, 0.0)
        cP = statep.tile([128, cp], F32, tag="cP")
        gP = statep.tile([128, cp], F32, tag="gP")
        t1P = statep.tile([128, cp], F32, tag="t1P")
        uPs = [statep.tile([128, cp], F32, name=f"uP{i}", tag=f"uP{i}")
               for i in range(LAGP + 6)]
        # 1-element scratch: "touch" ops read freshly-DMA'd tiles so the
        # DMA-completion wait lands on the touch, not on a compute op (the
        # compute ISA structs have one sync-wait slot and the serial RAW
        # chain already consumes it).
        touch_w = statep.tile([128, 1], F32, tag="touch_w")
        nc.vector.tensor_copy(out=touch_w, in_=w1t[:, :1])
        touch_wp = statep.tile([128, 1], F32, tag="touch_wp")
        nc.gpsimd.tensor_copy(out=touch_wp, in_=w1t[:, :1])

        # step t -> (out_tile, offset); Pool's spike for step t is emitted one
        # iteration late so it fills the dependency bubble after add(t+1)
        # instead of sitting between add(t) and its dependents
        spike_slot = {}
        pend_out = {}

        def emit_spikeP(tp):
            otile, offp, icp = spike_slot.pop(tp)
            nc.gpsimd.tensor_scalar(out=otile[:, offp, cd:],
                                    in0=uPs[tp % len(uPs)],
                                    scalar1=1.0, scalar2=None, op0=OP.is_gt)
            if icp is not None:
                nc.sync.dma_start(out=out_aps[icp], in_=pend_out.pop(icp))

        s = 0
        for ic, csz in enumerate(csizes):
            if ic == 0:
                xin = xin0
            else:
                xin = xinp.tile([128, csz, C], F32, tag="xin")
                nc.sync.dma_start(out=xin, in_=x_ap[:, s:s + csz, :])
            tchD = statep.tile([128, 1], F32, tag=f"touchD{ic}")
            nc.vector.tensor_copy(out=tchD, in_=xin[:, 0, :1])
            tchP = statep.tile([128, 1], F32, tag=f"touchP{ic}")
            nc.gpsimd.tensor_copy(out=tchP, in_=xin[:, 0, cd:cd + 1])
            out_tile = outp.tile([128, csz, C], F16, tag="out")
            pend_out[ic] = out_tile

            for off in range(csz):
                t = s + off
                uA = uAs[t % 6]
                uB = uBs[t % 6]
                uP = uPs[t % len(uPs)]
                last = (t == t_in - 1)   # final z is never consumed
                # DVE slice: 3-op chain x 2 interleaved half-chains
                nc.vector.tensor_add(out=uA, in0=xin[:, off, :ca], in1=zA)
                nc.vector.tensor_add(out=uB, in0=xin[:, off, ca:cd], in1=zB)
                if not last:
                    nc.vector.scalar_tensor_tensor(
                        out=t1A, in0=uA, scalar=1.0, in1=uA,
                        op0=OP.is_lt, op1=OP.mult)
                    nc.vector.scalar_tensor_tensor(
                        out=t1B, in0=uB, scalar=1.0, in1=uB,
                        op0=OP.is_lt, op1=OP.mult)
                    nc.vector.scalar_tensor_tensor(
                        out=zA, in0=t1A, scalar=0.0, in1=w1t[:, :ca],
                        op0=OP.max, op1=OP.mult)
                    nc.vector.scalar_tensor_tensor(
                        out=zB, in0=t1B, scalar=0.0, in1=w1t[:, ca:cd],
                        op0=OP.max, op1=OP.mult)
                # Pool slice: 6-op chain (walrus rejects STT on Pool); Pool
                # emits its own spikes so the two chains never couple through
                # the Act queue, and one step late so the spike op fills the
                # z->add dependency bubble
                nc.gpsimd.tensor_tensor(out=uP, in0=xin[:, off, cd:], in1=zP,
                                        op=OP.add)
                spike_slot[t] = (out_tile, off,
                                 ic if off == csz - 1 else None)
                if t - 1 in spike_slot:
                    emit_spikeP(t - 1)
                if not last:
                    nc.gpsimd.tensor_scalar(out=cP, in0=uP, scalar1=1.0,
                                            scalar2=0.0, op0=OP.min,
                                            op1=OP.max)
                    nc.gpsimd.tensor_scalar(out=gP, in0=uP, scalar1=1.0,
                                            scalar2=None, op0=OP.is_lt)
                    nc.gpsimd.tensor_tensor(out=t1P, in0=cP, in1=gP,
                                            op=OP.mult)
                    nc.gpsimd.tensor_tensor(out=zP, in0=t1P,
                                            in1=w1t[:, cd:], op=OP.mult)
                # spike carrier: s = Sign(u - 1) on Act for the DVE slice.
                # On the very last step DVE is idle and Act lags ~a step, so
                # emit the final spikes on DVE to shorten the drain path.
                if last:
                    nc.vector.tensor_scalar(out=out_tile[:, off, :ca],
                                            in0=uA, scalar1=1.0,
                                            scalar2=None, op0=OP.is_gt)
                    nc.vector.tensor_scalar(out=out_tile[:, off, ca:cd],
                                            in0=uB, scalar1=1.0,
                                            scalar2=None, op0=OP.is_gt)
                else:
                    nc.scalar.activation(out=out_tile[:, off, :ca], in_=uA,
                                         func=AF.Sign, bias=bias[:],
                                         scale=1.0)
                    nc.scalar.activation(out=out_tile[:, off, ca:cd], in_=uB,
                                         func=AF.Sign, bias=bias[:],
                                         scale=1.0)

            if ic < len(csizes) - 1:
                # release markers: a 4-byte write by each engine after its
                # last read of the slot, so the refill DMA's waits resolve to
                # one sem per engine (transitively covering the old DMA).
                nc.vector.memset(xin[:, 0, :1], 0.0)
                nc.gpsimd.memset(xin[:, 0, cd:cd + 1], 0.0)
            s += csz

        emit_spikeP(t_in - 1)


def _legalize_waits(nc):
    """Walrus accepts at most one sync wait on compute/DMA ISA structs.
    Split extra waits onto standalone EventSemaphore instructions inserted
    just before, on the same engine queue (identical blocking semantics)."""
    import bass_rust
    skip = ("InstEventSemaphore",)
    for f in nc.m.functions:
        for bb in f.blocks:
            insts = bb.instructions
            k = 0
            while k < len(insts):
                i = insts[k]
                si = i.sync_info
                if (si is not None and si.on_wait and len(si.on_wait) > 1
                        and type(i).__name__ not in skip):
                    waits = list(si.on_wait)
                    for j, w in enumerate(waits[:-1]):
                        ev = mybir.InstEventSemaphore(
                            name=f"{i.name}-evw{j}",
                            engine=i.engine,
                            ins=[], outs=[],
                            sync_info=bass_rust.SyncInfo(
                                on_wait=[w], on_update=[]),
                        )
                        insts.insert(k, ev)
                        k += 1
                    i.sync_info = bass_rust.SyncInfo(
                        on_wait=[waits[-1]], on_update=si.on_update)
                k += 1


def build(t_in=T_IN, cd=CD):
    nc = bass.Bass("TRN2", target_bir_lowering=False, debug=False,
                   enable_asserts=False, num_devices=NCORES)
    x_d = nc.dram_tensor("x_local", [B, t_in, C], F32, kind="ExternalInput")
    w1_d = nc.dram_tensor("w1b", [128, C], F32, kind="ExternalInput")
    out_ds = [
        nc.dram_tensor(f"spikes{i}", [B, csz, C], F16, kind="ExternalOutput")
        for i, csz in enumerate(in_chunk_sizes(t_in))
    ]
    with TileContext(nc) as tc:
        lif_body(tc, [d[:] for d in out_ds], x_d[:], w1_d[:], t_in, cd)
    _legalize_waits(nc)
    return nc


def _core_start(k):
    """Global t of core k's first OUTPUT step."""
    return 0 if k == 0 else T_IN + (k - 1) * L


def _host_repair(out, x, w1):
    """Exactly recompute lanes whose warmup windows lack a reset/clamp
    certificate at some core boundary, and patch them into `out`."""
    missing = np.zeros((B, C), bool)
    for k in range(1, NCORES):
        t0 = _core_start(k)
        win = x[:, t0 - WARM:t0, :]
        cert = ((win >= np.float32(1.0)) |
                (win <= -w1[None, None, :])).any(axis=1)
        missing |= ~cert
    if not missing.any():
        return 0
    bb, cc = np.nonzero(missing)
    xs = x[bb, :, cc]                     # [R, T]
    a = w1[cc]                            # [R]
    zz = np.zeros(len(bb), np.float32)
    one = np.float32(1.0)
    zero = np.float32(0.0)
    sp = np.empty((len(bb), T), np.float32)
    for t in range(T):
        u = (xs[:, t] + zz).astype(np.float32)
        t1 = ((u < one).astype(np.float32) * u).astype(np.float32)
        zz = (np.maximum(t1, zero) * a).astype(np.float32)
        sp[:, t] = (u > one).astype(np.float32)
    out[bb, :, cc] = sp
    return len(bb)


def kernel(x, w_leak):
    global LAST_RESULTS
    x = np.ascontiguousarray(np.asarray(x), dtype=np.float32)
    w_leak = np.ascontiguousarray(np.asarray(w_leak), dtype=np.float32)
    w1 = (np.float32(1.0) - w_leak).astype(np.float32)       # [C]
    w1b = np.ascontiguousarray(np.broadcast_to(w1[None, :], (128, C)),
                               dtype=np.float32)

    in_maps = []
    for k in range(NCORES):
        t0 = _core_start(k)
        lo = t0 - (0 if k == 0 else WARM)
        in_maps.append({
            "x_local": np.ascontiguousarray(x[:, lo:lo + T_IN, :]),
            "w1b": w1b,
        })

    nc = build()
    res = bass_utils.run_bass_kernel_spmd(
        nc, in_maps, core_ids=list(range(NCORES)), trace=TRACE)
    LAST_RESULTS = res
    nchunks = len(in_chunk_sizes(T_IN))
    out = np.empty((B, T, C), np.float32)
    for k in range(NCORES):
        sgn = np.concatenate(
            [res.results[k][f"spikes{i}"] for i in range(nchunks)], axis=1)
        skip = 0 if k == 0 else WARM
        t0 = _core_start(k)
        n = T_IN - skip
        out[:, t0:t0 + n, :] = (sgn[:, skip:, :] > 0).astype(np.float32)
    _host_repair(out, x, w1)
    return out


# revision 31
# speedup vs baseline: 1.0065x; 1.0006x over previous
"""LIF activation (hard-reset leaky integrate-and-fire) on 8 Trainium2 cores.

Math (per lane, per step t):
    u_t   = x_t + z_{t-1}
    Vm_t  = relu(u_t)
    keep  = 1{Vm_t < 1}  == 1{u_t < 1}
    z_t   = (1 - w_leak) * Vm_t * keep     (carried state, pre-scaled)
    spike = 1{u_t > 1}                     (strict >, the output)

Engines (all exact f32; lanes are independent, so the channel dim is split
between two engines that each run their own serial recurrence):
  DVE   (channels [0:CD]):   u = x + z                  tensor_tensor add
                             t1 = 1{u<1} * u            scalar_tensor_tensor
                             z = max(t1,0) * W1         scalar_tensor_tensor
  Pool  (channels [CD:C]):   u = x + z                  tensor_tensor add
                             s = 1{u>1}                 tensor_scalar is_gt
                             c = clip(u,0,1)            tensor_scalar min,max
                             g = 1{u<1}                 tensor_scalar is_lt
                             t1 = c * g                 tensor_tensor mult
                             z = t1 * W1                tensor_tensor mult
  Act   (DVE's channels):    s = Sign(u - 1)            activation, off-path
The spike is recovered on the host as (s > 0); Sign/is_gt are exact in f32,
so the result is bit-identical to the f32 reference. Emitting s in fp16
(values in {-1, 0, 1} are exact) halves the output DMA.

Sharding: time is split unevenly across the 8 cores: every core computes
T_IN = 132 steps; core 0 outputs all 132 (starts from the true z=0), cores
1-7 warm up speculatively for WARM=8 steps from z=0 and output 124 steps.
Hard resets make any starting state collapse: if within the warmup window a
lane sees x_t >= 1 (forced reset for any state) or x_t <= -(1-w_leak)
(forced relu clamp), the warmed-up state is provably bit-exact. Lanes with
no such certificate in some window are recomputed exactly on the host and
patched in, so the result is exact for any input.
"""
import numpy as np
import sys

for _p in ("/opt/trn_rl_repo",):
    if _p not in sys.path:
        sys.path.append(_p)

import concourse.bass as bass
import concourse.mybir as mybir
from concourse.tile import TileContext
from concourse import bass_utils

F32 = mybir.dt.float32
F16 = mybir.dt.float16
OP = mybir.AluOpType
AF = mybir.ActivationFunctionType

B, T, C = 128, 1000, 512
NCORES = 8
WARM = 8                  # speculative warmup steps (cores 1..7)
T_IN = (T + (NCORES - 1) * WARM) // NCORES   # 132 compute steps per core
L = T_IN - WARM           # 124 output steps per core 1..7; core 0 emits T_IN
CHUNK = 16                # time steps per DMA chunk
CHUNK0 = 4                # ramp unit for leading/trailing chunks
CD = 424                  # channels on DVE; C - CD on Pool
LAGP = 4                  # uP rotation depth headroom (see uPs below)

TRACE = False             # kept for test.py compatibility (no NTFF here)
LAST_RESULTS = None       # BassKernelResults of the last run


def in_chunk_sizes(t_in=T_IN, chunk=CHUNK, chunk0=CHUNK0):
    """Ramped chunk plan: small leading chunks so compute starts before the
    full-size DMAs land, and small trailing chunks so the final stores are
    short. E.g. t_in=132 -> [1, 2, 4, 8, 16*6, 13, 4, 4]."""
    sizes = [1, chunk0 // 2, chunk0, 2 * chunk0] if chunk0 else []
    o = sum(sizes)
    while t_in - o > chunk + 2 * chunk0:
        sizes.append(chunk)
        o += chunk
    rem = t_in - o
    if rem > 3 * chunk0:
        sizes += [rem - 3 * chunk0, chunk0, chunk0, chunk0 // 2,
                  chunk0 // 4, chunk0 // 4]
    elif rem:
        sizes.append(rem)
    return sizes


def lif_body(tc, out_aps, x_ap, w1_ap, t_in=T_IN, cd=CD):
    """Emit the per-core LIF program.

    out_aps: list of [B, csz_i, C] f16 DRAM tensors, one per chunk
    x_ap: [B, t_in, C] f32 DRAM
    w1_ap:  [128, C] f32 DRAM (1 - w_leak, pre-broadcast over partitions)
    """
    nc = tc.nc
    cp = C - cd
    csizes = in_chunk_sizes(t_in)
    with tc.tile_pool(name="const", bufs=1) as constp, \
         tc.tile_pool(name="state", bufs=1) as statep, \
         tc.tile_pool(name="xin", bufs=3) as xinp, \
         tc.tile_pool(name="outs", bufs=3) as outp:
        # first x chunk's DMA goes ahead of everything else on the (serialized)
        # DMA engines — it gates the first compute step; w1 is needed two ops
        # later and can follow
        xin0 = xinp.tile([128, csizes[0], C], F32, tag="xin")
        nc.sync.dma_start(out=xin0, in_=x_ap[:, 0:csizes[0], :])
        w1t = constp.tile([128, C], F32)
        nc.sync.dma_start(out=w1t, in_=w1_ap)
        bias = constp.tile([128, 1], F32)
        nc.vector.memset(bias, -1.0)
        # --- DVE-slice state: two independent half-chains (A|B) interleaved
        # so each op's RAW producer is two instructions back and the DVE
        # pipeline's side-effect latency is hidden ---
        ca = cd // 2
        zA = statep.tile([128, ca], F32)
        nc.vector.memset(zA, 0.0)
        zB = statep.tile([128, cd - ca], F32)
        nc.vector.memset(zB, 0.0)
        t1A = statep.tile([128, ca], F32, tag="t1A")
        t1B = statep.tile([128, cd - ca], F32, tag="t1B")
        # Act reads u; rotate deep so the cross-engine WAR wait is stale by
        # the time the DVE add reuses a slot
        uAs = [statep.tile([128, ca], F32, name=f"uA{i}", tag=f"uA{i}")
               for i in range(6)]
        uBs = [statep.tile([128, cd - ca], F32, name=f"uB{i}", tag=f"uB{i}")
               for i in range(6)]
        # --- Pool-slice state: Pool's spike op reads uP one step late, so
        # rotate uP a few slots deep ---
        zP = statep.tile([128, cp], F32)
        nc.gpsimd.memset(zP, 0.0)
        cP = statep.tile([128, cp], F32, tag="cP")
        gP = statep.tile([128, cp], F32, tag="gP")
        t1P = statep.tile([128, cp], F32, tag="t1P")
        uPs = [statep.tile([128, cp], F32, name=f"uP{i}", tag=f"uP{i}")
               for i in range(LAGP + 6)]
        # 1-element scratch: "touch" ops read freshly-DMA'd tiles so the
        # DMA-completion wait lands on the touch, not on a compute op (the
        # compute ISA structs have one sync-wait slot and the serial RAW
        # chain already consumes it).
        touch_w = statep.tile([128, 1], F32, tag="touch_w")
        nc.vector.tensor_copy(out=touch_w, in_=w1t[:, :1])
        touch_wp = statep.tile([128, 1], F32, tag="touch_wp")
        nc.gpsimd.tensor_copy(out=touch_wp, in_=w1t[:, :1])

        # step t -> (out_tile, offset); Pool's spike for step t is emitted one
        # iteration late so it fills the dependency bubble after add(t+1)
        # instead of sitting between add(t) and its dependents
        spike_slot = {}
        pend_out = {}

        def emit_spikeP(tp):
            otile, offp, icp = spike_slot.pop(tp)
            nc.gpsimd.tensor_scalar(out=otile[:, offp, cd:],
                                    in0=uPs[tp % len(uPs)],
                                    scalar1=1.0, scalar2=None, op0=OP.is_gt)
            if icp is not None:
                nc.sync.dma_start(out=out_aps[icp], in_=pend_out.pop(icp))

        s = 0
        for ic, csz in enumerate(csizes):
            if ic == 0:
                xin = xin0
            else:
                xin = xinp.tile([128, csz, C], F32, tag="xin")
                nc.sync.dma_start(out=xin, in_=x_ap[:, s:s + csz, :])
            tchD = statep.tile([128, 1], F32, tag=f"touchD{ic}")
            nc.vector.tensor_copy(out=tchD, in_=xin[:, 0, :1])
            tchP = statep.tile([128, 1], F32, tag=f"touchP{ic}")
            nc.gpsimd.tensor_copy(out=tchP, in_=xin[:, 0, cd:cd + 1])
            out_tile = outp.tile([128, csz, C], F16, tag="out")
            pend_out[ic] = out_tile

            for off in range(csz):
                t = s + off
                uA = uAs[t % 6]
                uB = uBs[t % 6]
                uP = uPs[t % len(uPs)]
                last = (t == t_in - 1)   # final z is never consumed
                # DVE slice: 3-op chain x 2 interleaved half-chains
                nc.vector.tensor_add(out=uA, in0=xin[:, off, :ca], in1=zA)
                nc.vector.tensor_add(out=uB, in0=xin[:, off, ca:cd], in1=zB)
                if not last:
                    nc.vector.scalar_tensor_tensor(
                        out=t1A, in0=uA, scalar=1.0, in1=uA,
                        op0=OP.is_lt, op1=OP.mult)
                    nc.vector.scalar_tensor_tensor(
                        out=t1B, in0=uB, scalar=1.0, in1=uB,
                        op0=OP.is_lt, op1=OP.mult)
                    nc.vector.scalar_tensor_tensor(
                        out=zA, in0=t1A, scalar=0.0, in1=w1t[:, :ca],
                        op0=OP.max, op1=OP.mult)
                    nc.vector.scalar_tensor_tensor(
                        out=zB, in0=t1B, scalar=0.0, in1=w1t[:, ca:cd],
                        op0=OP.max, op1=OP.mult)
                # Pool slice: 6-op chain (walrus rejects STT on Pool); Pool
                # emits its own spikes so the two chains never couple through
                # the Act queue, and one step late so the spike op fills the
                # z->add dependency bubble
                nc.gpsimd.tensor_tensor(out=uP, in0=xin[:, off, cd:], in1=zP,
                                        op=OP.add)
                spike_slot[t] = (out_tile, off,
                                 ic if off == csz - 1 else None)
                if t - 1 in spike_slot:
                    emit_spikeP(t - 1)
                if not last:
                    nc.gpsimd.tensor_scalar(out=cP, in0=uP, scalar1=1.0,
                                            scalar2=0.0, op0=OP.min,
                                            op1=OP.max)
                    nc.gpsimd.tensor_scalar(out=gP, in0=uP, scalar1=1.0,
                                            scalar2=None, op0=OP.is_lt)
                    nc.gpsimd.tensor_tensor(out=t1P, in0=cP, in1=gP,
                                            op=OP.mult)
                    nc.gpsimd.tensor_tensor(out=zP, in0=t1P,
                                            in1=w1t[:, cd:], op=OP.mult)
                # spike carrier: s = Sign(u - 1) on Act for the DVE slice.
                # On the very last step DVE is idle and Act lags ~a step, so
                # emit the final spikes on DVE to shorten the drain path.
                if last:
                    nc.vector.tensor_scalar(out=out_tile[:, off, :ca],
                                            in0=uA, scalar1=1.0,
                                            scalar2=None, op0=OP.is_gt)
                    nc.vector.tensor_scalar(out=out_tile[:, off, ca:cd],
                                            in0=uB, scalar1=1.0,
                                            scalar2=None, op0=OP.is_gt)
                else:
                    nc.scalar.activation(out=out_tile[:, off, :ca], in_=uA,
                                         func=AF.Sign, bias=bias[:],
                                         scale=1.0)
                    nc.scalar.activation(out=out_tile[:, off, ca:cd], in_=uB,
                                         func=AF.Sign, bias=bias[:],
                                         scale=1.0)

            if ic < len(csizes) - 1:
                # release markers: a 4-byte write by each engine after its
                # last read of the slot, so the refill DMA's waits resolve to
                # one sem per engine (transitively covering the old DMA).
                nc.vector.memset(xin[:, 0, :1], 0.0)
                nc.gpsimd.memset(xin[:, 0, cd:cd + 1], 0.0)
            s += csz

        emit_spikeP(t_in - 1)


def _legalize_waits(nc):
    """Walrus accepts at most one sync wait on compute/DMA ISA structs.
    Split extra waits onto standalone EventSemaphore instructions inserted
    just before, on the same engine queue (identical blocking semantics)."""
    import bass_rust
    skip = ("InstEventSemaphore",)
    for f in nc.m.functions:
        for bb in f.blocks:
            insts = bb.instructions
            k = 0
            while k < len(insts):
                i = insts[k]
                si = i.sync_info
                if (si is not None and si.on_wait and len(si.on_wait) > 1
                        and type(i).__name__ not in skip):
                    waits = list(si.on_wait)
                    for j, w in enumerate(waits[:-1]):
                        ev = mybir.InstEventSemaphore(
                            name=f"{i.name}-evw{j}",
                            engine=i.engine,
                            ins=[], outs=[],
                            sync_info=bass_rust.SyncInfo(
                                on_wait=[w], on_update=[]),
                        )
                        insts.insert(k, ev)
                        k += 1
                    i.sync_info = bass_rust.SyncInfo(
                        on_wait=[waits[-1]], on_update=si.on_update)
                k += 1


def build(t_in=T_IN, cd=CD):
    nc = bass.Bass("TRN2", target_bir_lowering=False, debug=False,
                   enable_asserts=False, num_devices=NCORES)
    x_d = nc.dram_tensor("x_local", [B, t_in, C], F32, kind="ExternalInput")
    w1_d = nc.dram_tensor("w1b", [128, C], F32, kind="ExternalInput")
    out_ds = [
        nc.dram_tensor(f"spikes{i}", [B, csz, C], F16, kind="ExternalOutput")
        for i, csz in enumerate(in_chunk_sizes(t_in))
    ]
    with TileContext(nc) as tc:
        lif_body(tc, [d[:] for d in out_ds], x_d[:], w1_d[:], t_in, cd)
    _legalize_waits(nc)
    return nc


def _core_start(k):
    """Global t of core k's first OUTPUT step."""
    return 0 if k == 0 else T_IN + (k - 1) * L


def _host_repair(out, x, w1):
    """Exactly recompute lanes whose warmup windows lack a reset/clamp
    certificate at some core boundary, and patch them into `out`."""
    missing = np.zeros((B, C), bool)
    for k in range(1, NCORES):
        t0 = _core_start(k)
        win = x[:, t0 - WARM:t0, :]
        cert = ((win >= np.float32(1.0)) |
                (win <= -w1[None, None, :])).any(axis=1)
        missing |= ~cert
    if not missing.any():
        return 0
    bb, cc = np.nonzero(missing)
    xs = x[bb, :, cc]                     # [R, T]
    a = w1[cc]                            # [R]
    zz = np.zeros(len(bb), np.float32)
    one = np.float32(1.0)
    zero = np.float32(0.0)
    sp = np.empty((len(bb), T), np.float32)
    for t in range(T):
        u = (xs[:, t] + zz).astype(np.float32)
        t1 = ((u < one).astype(np.float32) * u).astype(np.float32)
        zz = (np.maximum(t1, zero) * a).astype(np.float32)
        sp[:, t] = (u > one).astype(np.float32)
    out[bb, :, cc] = sp
    return len(bb)


def kernel(x, w_leak):
    global LAST_RESULTS
    x = np.ascontiguousarray(np.asarray(x), dtype=np.float32)
    w_leak = np.ascontiguousarray(np.asarray(w_leak), dtype=np.float32)
    w1 = (np.float32(1.0) - w_leak).astype(np.float32)       # [C]
    w1b = np.ascontiguousarray(np.broadcast_to(w1[None, :], (128, C)),
                               dtype=np.float32)

    in_maps = []
    for k in range(NCORES):
        t0 = _core_start(k)
        lo = t0 - (0 if k == 0 else WARM)
        in_maps.append({
            "x_local": np.ascontiguousarray(x[:, lo:lo + T_IN, :]),
            "w1b": w1b,
        })

    nc = build()
    res = bass_utils.run_bass_kernel_spmd(
        nc, in_maps, core_ids=list(range(NCORES)), trace=TRACE)
    LAST_RESULTS = res
    nchunks = len(in_chunk_sizes(T_IN))
    out = np.empty((B, T, C), np.float32)
    for k in range(NCORES):
        sgn = np.concatenate(
            [res.results[k][f"spikes{i}"] for i in range(nchunks)], axis=1)
        skip = 0 if k == 0 else WARM
        t0 = _core_start(k)
        n = T_IN - skip
        out[:, t0:t0 + n, :] = (sgn[:, skip:, :] > 0).astype(np.float32)
    _host_repair(out, x, w1)
    return out
